# revision 16
# baseline (speedup 1.0000x reference)
"""MinLSTM cell kernel for Trainium2, 8 NeuronCores.

Problem: B=4, S=4096, D=1024, H=1024.
  gates = x @ W.T + b            (B,S,4H)
  f' = sigmoid(-(softplus(-fg) - softplus(-ig)))   (normalized log-space gates)
  i' = 1 - f'
  g  = max(cs + 0.5, sigmoid(cs))                  (== x+0.5 for x>=0 else sigmoid)
  c_t = f'_t * c_{t-1} + i'_t * g_t                (sequential scan along S)
  out = sigmoid(og) * c

Sharding: (batch, H-half) -> 8 shards; each core handles 1 batch x 512 hidden
units over the full sequence, so the scan has no cross-core dependency.

Device layout: everything lives as [hidden (partitions), seq (free)] so the
DVE TensorTensorScan instruction runs the recurrence natively along the free
axis. Host pre-transposes x and W per shard (so the PE does zero transposes)
and transposes the [H,S] device output back to [S,H] at unshard time.
Matmuls run as float32r (fp22 mantissa truncation, full PE rate).
"""
import sys

sys.path.insert(0, "/opt/trn_rl_repo")

import numpy as np

import concourse.tile as tile
from concourse import bacc, mybir
from concourse.bass_utils import run_bass_kernel_spmd

B, S, D, H = 4, 4096, 1024, 1024
NCORES = 8
HSH = 512            # hidden units per core
CHUNK = 512          # sequence positions per chunk
NCHUNK = S // CHUNK  # 8
MT = 4 * HSH // 128  # 16 m-tiles (4 gates x 4 h-tiles)
KT = D // 128        # 8 k-tiles
JT = HSH // 128      # 4 h-tiles per core

F32 = mybir.dt.float32
F32R = mybir.dt.float32r
ACT = mybir.ActivationFunctionType
ALU = mybir.AluOpType

_prog_cache = {}


def _build_program():
    nc = bacc.Bacc("TRN2", target_bir_lowering=False, debug=False, num_devices=NCORES)

    xt_d = nc.dram_tensor("xt", [D, S], F32, kind="ExternalInput").ap()
    wt_d = nc.dram_tensor("wt", [D, 4 * HSH], F32, kind="ExternalInput").ap()
    bias_d = nc.dram_tensor("bias", [128, 3 * MT], F32, kind="ExternalInput").ap()
    c0_d = nc.dram_tensor("c0", [128, JT], F32, kind="ExternalInput").ap()
    outT_d = nc.dram_tensor("outT", [HSH, S], F32, kind="ExternalOutput").ap()
    clast_d = nc.dram_tensor("clast", [128, JT], F32, kind="ExternalOutput").ap()

    with tile.TileContext(nc) as tc:
        with tc.tile_pool(name="const", bufs=1) as const_pool, \
             tc.tile_pool(name="wtp", bufs=1) as wt_pool, \
             tc.tile_pool(name="xin", bufs=2) as x_pool, \
             tc.tile_pool(name="ew", bufs=3) as ew_pool, \
             tc.tile_pool(name="cp", bufs=2) as c_pool, \
             tc.tile_pool(name="gps", bufs=8, space="PSUM") as g_pool:

            # Stationary weights, split per (k-block, m-half) and DMA'd in
            # the order chunk 0 consumes them, interleaved with chunk-0 x
            # pieces, so the PE starts ~1 us in and follows the W stream.
            wt_view = wt_d.rearrange("(dk p) m -> p dk m", p=128).bitcast(F32R)
            MH = 2 * HSH  # m-half width
            wt_sb = [[None, None] for _ in range(KT)]
            x0_sb = []
            bias_sb = None
            for dk in range(KT):
                xp = x_pool.tile([128, CHUNK], F32R, tag=f"xt{dk}", name=f"x0_{dk}")
                nc.sync.dma_start(xp[:], xt_d[dk * 128 : (dk + 1) * 128, 0:CHUNK].bitcast(F32R))
                x0_sb.append(xp)
                if dk == 0:
                    # first W arrival split smaller so the PE starts sooner
                    wa = wt_pool.tile([128, MH // 2], F32R, tag="wt0h0a", name="wt0h0a")
                    nc.sync.dma_start(wa[:], wt_view[:, 0, 0 : MH // 2])
                    wb = wt_pool.tile([128, MH // 2], F32R, tag="wt0h0b", name="wt0h0b")
                    nc.sync.dma_start(wb[:], wt_view[:, 0, MH // 2 : MH])
                    wt_sb[0][0] = (wa, wb)
                    # small constants ride along after the first pieces
                    bias_sb = const_pool.tile([128, 3 * MT], F32)
                    nc.sync.dma_start(bias_sb[:], bias_d)
                    c0_sb = const_pool.tile([128, JT], F32)
                    nc.sync.dma_start(c0_sb[:], c0_d)
                else:
                    w1 = wt_pool.tile([128, MH], F32R, tag=f"wt{dk}h0", name=f"wt{dk}h0")
                    nc.sync.dma_start(w1[:], wt_view[:, dk, 0:MH])
                    wt_sb[dk][0] = w1
            for dk in range(KT):
                w2 = wt_pool.tile([128, MH], F32R, tag=f"wt{dk}h1", name=f"wt{dk}h1")
                nc.sync.dma_start(w2[:], wt_view[:, dk, MH : 2 * MH])
                wt_sb[dk][1] = w2
            clast_sb = const_pool.tile([128, JT], F32)

            def bcol(v, t):
                return bias_sb[:, v * MT + t : v * MT + t + 1]

            def emit_mm(pt, t, dk, xt_dk, cs):
                half, tm = divmod(t, 8)
                w = wt_sb[dk][half]
                if isinstance(w, tuple):
                    w = w[tm // 4]
                    tm = tm % 4
                nc.tensor.matmul(
                    pt[:, :cs],
                    lhsT=w[:, tm * 128 : (tm + 1) * 128],
                    rhs=xt_dk[:, :cs],
                    start=(dk == 0),
                    stop=(dk == KT - 1),
                )

            # Last chunk split in two to shorten the post-matmul tail;
            # emission order interleaves the last two chunks so only one
            # j-chain trails the final matmul.
            sizes = [CHUNK] * (NCHUNK - 1) + [CHUNK // 2, CHUNK // 2]
            offs = [sum(sizes[:k]) for k in range(len(sizes))]
            nchunks = len(sizes)
            xt_chunks = {0: x0_sb}
            aprod = {}   # i -> (fgates, irs)
            prev_c = [None] * JT

            def load_x(i):
                cs, s0 = sizes[i], offs[i]
                xt_sb = []
                for dk in range(KT):
                    xp = x_pool.tile([128, CHUNK], F32R, tag=f"xt{dk}")
                    nc.sync.dma_start(
                        xp[:, :cs],
                        xt_d[dk * 128 : (dk + 1) * 128, s0 : s0 + cs].bitcast(F32R),
                    )
                    xt_sb.append(xp)
                xt_chunks[i] = xt_sb

            def phase_a(i):
                cs = sizes[i]
                xt_sb = xt_chunks[i]
                pa = {}
                if i == 0:
                    # W streams in during chunk 0: issue dk-major so the PE
                    # follows the per-dk W arrivals instead of stalling on
                    # the full weight load inside one accumulation group.
                    for t in range(8):
                        pa[t] = g_pool.tile([128, CHUNK], F32, tag="g", name=f"pa{t}")
                    for dk in range(KT):
                        for t in range(8):
                            emit_mm(pa[t], t, dk, xt_sb[dk], cs)
                else:
                    for t in range(8):
                        pa[t] = g_pool.tile([128, CHUNK], F32, tag="g", name=f"pa{t}")
                        for dk in range(KT):
                            emit_mm(pa[t], t, dk, xt_sb[dk], cs)
                irs, fgates = [], []
                for j in range(JT):
                    ig_p, fg_p = pa[j], pa[JT + j]
                    si = ew_pool.tile([128, CHUNK], F32, tag="si", bufs=2)
                    nc.scalar.activation(
                        si[:, :cs], ig_p[:, :cs], ACT.Sigmoid, bias=bcol(0, j)
                    )
                    sf = ew_pool.tile([128, CHUNK], F32, tag="sf", bufs=2)
                    nc.scalar.activation(
                        sf[:, :cs], fg_p[:, :cs], ACT.Sigmoid, bias=bcol(0, JT + j)
                    )
                    den = ew_pool.tile([128, CHUNK], F32, tag="den", bufs=2)
                    nc.vector.tensor_add(den[:, :cs], sf[:, :cs], si[:, :cs])
                    rden = ew_pool.tile([128, CHUNK], F32, tag="rden", bufs=3)
                    nc.vector.reciprocal(rden[:, :cs], den[:, :cs])
                    fgate = ew_pool.tile([128, CHUNK], F32, tag="fgate", bufs=8)
                    nc.vector.tensor_mul(fgate[:, :cs], sf[:, :cs], rden[:, :cs])
                    # i' = s_i * rden, used below as v = i' * g
                    ir = ew_pool.tile([128, CHUNK], F32, tag="ir", bufs=8)
                    nc.vector.tensor_mul(ir[:, :cs], si[:, :cs], rden[:, :cs])
                    irs.append(ir)
                    fgates.append(fgate)
                    pa[j] = pa[JT + j] = None
                aprod[i] = (fgates, irs)

            def scan_part(i, j, cs_p, cs):
                # sg/gval/v/scan for one (chunk, h-tile) given the cs gate tile
                fgates, irs = aprod[i]
                sg = ew_pool.tile([128, CHUNK], F32, tag="sg", bufs=2)
                nc.scalar.activation(
                    sg[:, :cs], cs_p[:, :cs], ACT.Sigmoid, bias=bcol(0, 3 * JT + j)
                )
                # g = max(cs + b + 0.5, sigmoid(cs + b))
                gval = ew_pool.tile([128, CHUNK], F32, tag="gval", bufs=2)
                nc.vector.scalar_tensor_tensor(
                    gval[:, :cs], cs_p[:, :cs], bcol(2, 3 * JT + j), sg[:, :cs],
                    op0=ALU.add, op1=ALU.max,
                )
                v = ew_pool.tile([128, CHUNK], F32, tag="v", bufs=2)
                nc.vector.tensor_mul(v[:, :cs], irs[j][:, :cs], gval[:, :cs])
                c = c_pool.tile([128, CHUNK], F32, tag=f"c{j}")
                init = (
                    c0_sb[:, j : j + 1]
                    if i == 0
                    else prev_c[j][0][:, prev_c[j][1] - 1 : prev_c[j][1]]
                )
                nc.vector.tensor_tensor_scan(
                    c[:, :cs], fgates[j][:, :cs], v[:, :cs], initial=init,
                    op0=ALU.mult, op1=ALU.add,
                )
                prev_c[j] = (c, cs)
                return c

            def out_part(i, j, og_p, c, cs, s0, eng=None):
                sog = ew_pool.tile([128, CHUNK], F32, tag="sog", bufs=4)
                nc.scalar.activation(
                    sog[:, :cs], og_p[:, :cs], ACT.Sigmoid, bias=bcol(0, 2 * JT + j)
                )
                o = ew_pool.tile([128, CHUNK], F32, tag="o", bufs=6)
                (eng or nc.vector).tensor_mul(o[:, :cs], sog[:, :cs], c[:, :cs])
                nc.sync.dma_start(
                    outT_d[j * 128 : (j + 1) * 128, s0 : s0 + cs], o[:, :cs]
                )

            def phase_b(i):
                cs, s0 = sizes[i], offs[i]
                xt_sb = xt_chunks[i]
                last = i == nchunks - 1
                if not last:
                    for j in range(JT):
                        og_p = g_pool.tile([128, CHUNK], F32, tag="g")
                        for dk in range(KT):
                            emit_mm(og_p, 2 * JT + j, dk, xt_sb[dk], cs)
                        cs_p = g_pool.tile([128, CHUNK], F32, tag="g")
                        for dk in range(KT):
                            emit_mm(cs_p, 3 * JT + j, dk, xt_sb[dk], cs)
                        c = scan_part(i, j, cs_p, cs)
                        out_part(i, j, og_p, c, cs, s0)
                else:
                    # last chunk: all cs-gate matmuls first so the scan
                    # chains drain while the og matmuls still run
                    cands = []
                    for j in range(JT):
                        cs_p = g_pool.tile([128, CHUNK], F32, tag="g")
                        for dk in range(KT):
                            emit_mm(cs_p, 3 * JT + j, dk, xt_sb[dk], cs)
                        c = scan_part(i, j, cs_p, cs)
                        nc.vector.tensor_copy(
                            clast_sb[:, j : j + 1], c[:, cs - 1 : cs]
                        )
                        cands.append(c)
                    for j in range(JT):
                        og_p = g_pool.tile([128, CHUNK], F32, tag="g")
                        for dk in range(KT):
                            emit_mm(og_p, 2 * JT + j, dk, xt_sb[dk], cs)
                        # tail: the idle GPSIMD engine takes the final muls so
                        # the DVE backlog doesn't trail the last matmul
                        out_part(i, j, og_p, cands[j], cs, s0, eng=nc.gpsimd)
                aprod.pop(i)

            for i in range(1, nchunks):
                load_x(i)
            for i in range(nchunks - 2):
                phase_a(i)
                phase_b(i)
            # tail: A(n-2), A(n-1), B(n-2), B(n-1) so only one j-chain
            # trails the last matmul
            phase_a(nchunks - 2)
            phase_a(nchunks - 1)
            phase_b(nchunks - 2)
            phase_b(nchunks - 1)
            nc.sync.dma_start(clast_d, clast_sb[:])

    nc.compile()
    return nc


def _get_program():
    if "nc" not in _prog_cache:
        _prog_cache["nc"] = _build_program()
    return _prog_cache["nc"]


def _shard_inputs(x, W, b, c0):
    in_maps = []
    for core in range(NCORES):
        bi, hh = divmod(core, 2)
        xt = np.ascontiguousarray(x[bi].T)  # (D, S)
        rows = np.concatenate(
            [W[g * H + hh * HSH : g * H + (hh + 1) * HSH] for g in range(4)], axis=0
        )  # (4*HSH, D) ordered [ig; fg; og; cs]
        wt = np.ascontiguousarray(rows.T)  # (D, 4*HSH)
        bsh = np.concatenate(
            [b[g * H + hh * HSH : g * H + (hh + 1) * HSH] for g in range(4)]
        )  # (4*HSH,)
        bt = bsh.reshape(MT, 128).T  # [p, t]
        bias_arr = np.ascontiguousarray(
            np.concatenate([bt, -bt, bt + 0.5], axis=1), dtype=np.float32
        )  # (128, 3*MT)
        c0sh = c0[bi, 0, hh * HSH : (hh + 1) * HSH]
        c0_arr = np.ascontiguousarray(c0sh.reshape(JT, 128).T, dtype=np.float32)
        in_maps.append({"xt": xt, "wt": wt, "bias": bias_arr, "c0": c0_arr})
    return in_maps


def kernel(x, W, b, h0, c0, _trace=False):
    x = np.asarray(x, dtype=np.float32)
    W = np.asarray(W, dtype=np.float32)
    b = np.asarray(b, dtype=np.float32)
    c0 = np.asarray(c0, dtype=np.float32)

    nc = _get_program()
    in_maps = _shard_inputs(x, W, b, c0)
    try:
        res = run_bass_kernel_spmd(
            nc, in_maps, core_ids=list(range(NCORES)), trace=_trace
        )
    except (ModuleNotFoundError, ImportError):
        # NTFF profiling hooks unavailable in this environment
        res = run_bass_kernel_spmd(
            nc, in_maps, core_ids=list(range(NCORES)), trace=False
        )

    out = np.empty((B, S, H), dtype=np.float32)
    c_last = np.empty((B, 1, H), dtype=np.float32)
    for core in range(NCORES):
        bi, hh = divmod(core, 2)
        outT = res.results[core]["outT"]  # (HSH, S)
        out[bi, :, hh * HSH : (hh + 1) * HSH] = outT.T
        cl = res.results[core]["clast"]  # (128, JT): [p, j] -> h = j*128+p
        c_last[bi, 0, hh * HSH : (hh + 1) * HSH] = cl.T.reshape(-1)

    if _trace:
        kernel._last_results = res
    return out, out[:, -1:], c_last


# revision 17
# speedup vs baseline: 1.0007x; 1.0007x over previous
"""MinLSTM cell kernel for Trainium2, 8 NeuronCores.

Problem: B=4, S=4096, D=1024, H=1024.
  gates = x @ W.T + b            (B,S,4H)
  f' = sigmoid(-(softplus(-fg) - softplus(-ig)))   (normalized log-space gates)
  i' = 1 - f'
  g  = max(cs + 0.5, sigmoid(cs))                  (== x+0.5 for x>=0 else sigmoid)
  c_t = f'_t * c_{t-1} + i'_t * g_t                (sequential scan along S)
  out = sigmoid(og) * c

Sharding: (batch, H-half) -> 8 shards; each core handles 1 batch x 512 hidden
units over the full sequence, so the scan has no cross-core dependency.

Device layout: everything lives as [hidden (partitions), seq (free)] so the
DVE TensorTensorScan instruction runs the recurrence natively along the free
axis. Host pre-transposes x and W per shard (so the PE does zero transposes)
and transposes the [H,S] device output back to [S,H] at unshard time.
Matmuls run as float32r (fp22 mantissa truncation, full PE rate).
"""
import sys

sys.path.insert(0, "/opt/trn_rl_repo")

import numpy as np

import concourse.tile as tile
from concourse import bacc, mybir
from concourse.bass_utils import run_bass_kernel_spmd

B, S, D, H = 4, 4096, 1024, 1024
NCORES = 8
HSH = 512            # hidden units per core
CHUNK = 512          # sequence positions per chunk
NCHUNK = S // CHUNK  # 8
MT = 4 * HSH // 128  # 16 m-tiles (4 gates x 4 h-tiles)
KT = D // 128        # 8 k-tiles
JT = HSH // 128      # 4 h-tiles per core

F32 = mybir.dt.float32
F32R = mybir.dt.float32r
ACT = mybir.ActivationFunctionType
ALU = mybir.AluOpType

_prog_cache = {}


def _build_program():
    nc = bacc.Bacc("TRN2", target_bir_lowering=False, debug=False, num_devices=NCORES)

    xt_d = nc.dram_tensor("xt", [D, S], F32, kind="ExternalInput").ap()
    wt_d = nc.dram_tensor("wt", [D, 4 * HSH], F32, kind="ExternalInput").ap()
    bias_d = nc.dram_tensor("bias", [128, 3 * MT], F32, kind="ExternalInput").ap()
    c0_d = nc.dram_tensor("c0", [128, JT], F32, kind="ExternalInput").ap()
    outT_d = nc.dram_tensor("outT", [HSH, S], F32, kind="ExternalOutput").ap()
    clast_d = nc.dram_tensor("clast", [128, JT], F32, kind="ExternalOutput").ap()

    with tile.TileContext(nc) as tc:
        with tc.tile_pool(name="const", bufs=1) as const_pool, \
             tc.tile_pool(name="wtp", bufs=1) as wt_pool, \
             tc.tile_pool(name="xin", bufs=2) as x_pool, \
             tc.tile_pool(name="ew", bufs=3) as ew_pool, \
             tc.tile_pool(name="cp", bufs=2) as c_pool, \
             tc.tile_pool(name="gps", bufs=8, space="PSUM") as g_pool:

            # Stationary weights, split per (k-block, m-half) and DMA'd in
            # the order chunk 0 consumes them, interleaved with chunk-0 x
            # pieces, so the PE starts ~1 us in and follows the W stream.
            wt_view = wt_d.rearrange("(dk p) m -> p dk m", p=128).bitcast(F32R)
            MH = 2 * HSH  # m-half width
            wt_sb = [[None, None] for _ in range(KT)]
            x0_sb = []
            bias_sb = None
            for dk in range(KT):
                xp = x_pool.tile([128, CHUNK], F32R, tag=f"xt{dk}", name=f"x0_{dk}")
                nc.sync.dma_start(xp[:], xt_d[dk * 128 : (dk + 1) * 128, 0:CHUNK].bitcast(F32R))
                x0_sb.append(xp)
                if dk == 0:
                    # first W arrival split smaller so the PE starts sooner
                    wa = wt_pool.tile([128, MH // 2], F32R, tag="wt0h0a", name="wt0h0a")
                    nc.sync.dma_start(wa[:], wt_view[:, 0, 0 : MH // 2])
                    wb = wt_pool.tile([128, MH // 2], F32R, tag="wt0h0b", name="wt0h0b")
                    nc.sync.dma_start(wb[:], wt_view[:, 0, MH // 2 : MH])
                    wt_sb[0][0] = (wa, wb)
                    # small constants ride along after the first pieces
                    bias_sb = const_pool.tile([128, 3 * MT], F32)
                    nc.sync.dma_start(bias_sb[:], bias_d)
                    c0_sb = const_pool.tile([128, JT], F32)
                    nc.sync.dma_start(c0_sb[:], c0_d)
                else:
                    w1 = wt_pool.tile([128, MH], F32R, tag=f"wt{dk}h0", name=f"wt{dk}h0")
                    nc.sync.dma_start(w1[:], wt_view[:, dk, 0:MH])
                    wt_sb[dk][0] = w1
            for dk in range(KT):
                w2 = wt_pool.tile([128, MH], F32R, tag=f"wt{dk}h1", name=f"wt{dk}h1")
                nc.sync.dma_start(w2[:], wt_view[:, dk, MH : 2 * MH])
                wt_sb[dk][1] = w2
            clast_sb = const_pool.tile([128, JT], F32)

            def bcol(v, t):
                return bias_sb[:, v * MT + t : v * MT + t + 1]

            def emit_mm(pt, t, dk, xt_dk, cs):
                half, tm = divmod(t, 8)
                w = wt_sb[dk][half]
                if isinstance(w, tuple):
                    w = w[tm // 4]
                    tm = tm % 4
                nc.tensor.matmul(
                    pt[:, :cs],
                    lhsT=w[:, tm * 128 : (tm + 1) * 128],
                    rhs=xt_dk[:, :cs],
                    start=(dk == 0),
                    stop=(dk == KT - 1),
                )

            # Last chunk split in two to shorten the post-matmul tail;
            # emission order interleaves the last two chunks so only one
            # j-chain trails the final matmul.
            sizes = [CHUNK] * (NCHUNK - 1) + [CHUNK // 2, CHUNK // 2]
            offs = [sum(sizes[:k]) for k in range(len(sizes))]
            nchunks = len(sizes)
            xt_chunks = {0: x0_sb}
            aprod = {}   # i -> (fgates, irs)
            prev_c = [None] * JT

            def load_x(i):
                cs, s0 = sizes[i], offs[i]
                xt_sb = []
                for dk in range(KT):
                    xp = x_pool.tile([128, CHUNK], F32R, tag=f"xt{dk}")
                    nc.sync.dma_start(
                        xp[:, :cs],
                        xt_d[dk * 128 : (dk + 1) * 128, s0 : s0 + cs].bitcast(F32R),
                    )
                    xt_sb.append(xp)
                xt_chunks[i] = xt_sb

            def phase_a(i):
                cs = sizes[i]
                xt_sb = xt_chunks[i]
                pa = {}
                if i == 0:
                    # W streams in during chunk 0: issue dk-major so the PE
                    # follows the per-dk W arrivals instead of stalling on
                    # the full weight load inside one accumulation group.
                    for t in range(8):
                        pa[t] = g_pool.tile([128, CHUNK], F32, tag="g", name=f"pa{t}")
                    for dk in range(KT):
                        for t in range(8):
                            emit_mm(pa[t], t, dk, xt_sb[dk], cs)
                else:
                    for t in range(8):
                        pa[t] = g_pool.tile([128, CHUNK], F32, tag="g", name=f"pa{t}")
                        for dk in range(KT):
                            emit_mm(pa[t], t, dk, xt_sb[dk], cs)
                irs, fgates = [], []
                for j in range(JT):
                    ig_p, fg_p = pa[j], pa[JT + j]
                    si = ew_pool.tile([128, CHUNK], F32, tag="si", bufs=2)
                    nc.scalar.activation(
                        si[:, :cs], ig_p[:, :cs], ACT.Sigmoid, bias=bcol(0, j)
                    )
                    sf = ew_pool.tile([128, CHUNK], F32, tag="sf", bufs=2)
                    nc.scalar.activation(
                        sf[:, :cs], fg_p[:, :cs], ACT.Sigmoid, bias=bcol(0, JT + j)
                    )
                    den = ew_pool.tile([128, CHUNK], F32, tag="den", bufs=2)
                    nc.vector.tensor_add(den[:, :cs], sf[:, :cs], si[:, :cs])
                    rden = ew_pool.tile([128, CHUNK], F32, tag="rden", bufs=3)
                    nc.vector.reciprocal(rden[:, :cs], den[:, :cs])
                    fgate = ew_pool.tile([128, CHUNK], F32, tag="fgate", bufs=8)
                    nc.vector.tensor_mul(fgate[:, :cs], sf[:, :cs], rden[:, :cs])
                    # i' = s_i * rden, used below as v = i' * g
                    ir = ew_pool.tile([128, CHUNK], F32, tag="ir", bufs=8)
                    nc.vector.tensor_mul(ir[:, :cs], si[:, :cs], rden[:, :cs])
                    irs.append(ir)
                    fgates.append(fgate)
                    pa[j] = pa[JT + j] = None
                aprod[i] = (fgates, irs)

            def scan_part(i, j, cs_p, cs):
                # sg/gval/v/scan for one (chunk, h-tile) given the cs gate tile
                fgates, irs = aprod[i]
                sg = ew_pool.tile([128, CHUNK], F32, tag="sg", bufs=2)
                nc.scalar.activation(
                    sg[:, :cs], cs_p[:, :cs], ACT.Sigmoid, bias=bcol(0, 3 * JT + j)
                )
                # g = max(cs + b + 0.5, sigmoid(cs + b))
                gval = ew_pool.tile([128, CHUNK], F32, tag="gval", bufs=2)
                nc.vector.scalar_tensor_tensor(
                    gval[:, :cs], cs_p[:, :cs], bcol(2, 3 * JT + j), sg[:, :cs],
                    op0=ALU.add, op1=ALU.max,
                )
                v = ew_pool.tile([128, CHUNK], F32, tag="v", bufs=2)
                nc.vector.tensor_mul(v[:, :cs], irs[j][:, :cs], gval[:, :cs])
                c = c_pool.tile([128, CHUNK], F32, tag=f"c{j}")
                init = (
                    c0_sb[:, j : j + 1]
                    if i == 0
                    else prev_c[j][0][:, prev_c[j][1] - 1 : prev_c[j][1]]
                )
                nc.vector.tensor_tensor_scan(
                    c[:, :cs], fgates[j][:, :cs], v[:, :cs], initial=init,
                    op0=ALU.mult, op1=ALU.add,
                )
                prev_c[j] = (c, cs)
                return c

            def out_part(i, j, og_p, c, cs, s0, eng=None):
                sog = ew_pool.tile([128, CHUNK], F32, tag="sog", bufs=4)
                nc.scalar.activation(
                    sog[:, :cs], og_p[:, :cs], ACT.Sigmoid, bias=bcol(0, 2 * JT + j)
                )
                o = ew_pool.tile([128, CHUNK], F32, tag="o", bufs=6)
                (eng or nc.vector).tensor_mul(o[:, :cs], sog[:, :cs], c[:, :cs])
                nc.sync.dma_start(
                    outT_d[j * 128 : (j + 1) * 128, s0 : s0 + cs], o[:, :cs]
                )

            def phase_b(i):
                cs, s0 = sizes[i], offs[i]
                xt_sb = xt_chunks[i]
                last = i == nchunks - 1
                if i == 0:
                    # W h1 blocks are still streaming: dk-major across all 8
                    # phase-B tiles so the PE follows the arrivals instead of
                    # stalling a whole accumulation group on the last block.
                    pb = {}
                    for j in range(JT):
                        pb[2 * JT + j] = g_pool.tile(
                            [128, CHUNK], F32, tag="g", name=f"pbog{j}"
                        )
                        pb[3 * JT + j] = g_pool.tile(
                            [128, CHUNK], F32, tag="g", name=f"pbcs{j}"
                        )
                    for dk in range(KT):
                        for t in sorted(pb):
                            emit_mm(pb[t], t, dk, xt_sb[dk], cs)
                    for j in range(JT):
                        c = scan_part(i, j, pb[3 * JT + j], cs)
                        out_part(i, j, pb[2 * JT + j], c, cs, s0)
                elif not last:
                    for j in range(JT):
                        og_p = g_pool.tile([128, CHUNK], F32, tag="g")
                        for dk in range(KT):
                            emit_mm(og_p, 2 * JT + j, dk, xt_sb[dk], cs)
                        cs_p = g_pool.tile([128, CHUNK], F32, tag="g")
                        for dk in range(KT):
                            emit_mm(cs_p, 3 * JT + j, dk, xt_sb[dk], cs)
                        c = scan_part(i, j, cs_p, cs)
                        out_part(i, j, og_p, c, cs, s0)
                else:
                    # last chunk: all cs-gate matmuls first so the scan
                    # chains drain while the og matmuls still run
                    cands = []
                    for j in range(JT):
                        cs_p = g_pool.tile([128, CHUNK], F32, tag="g")
                        for dk in range(KT):
                            emit_mm(cs_p, 3 * JT + j, dk, xt_sb[dk], cs)
                        c = scan_part(i, j, cs_p, cs)
                        nc.vector.tensor_copy(
                            clast_sb[:, j : j + 1], c[:, cs - 1 : cs]
                        )
                        cands.append(c)
                    for j in range(JT):
                        og_p = g_pool.tile([128, CHUNK], F32, tag="g")
                        for dk in range(KT):
                            emit_mm(og_p, 2 * JT + j, dk, xt_sb[dk], cs)
                        # tail: the idle GPSIMD engine takes the final muls so
                        # the DVE backlog doesn't trail the last matmul
                        out_part(i, j, og_p, cands[j], cs, s0, eng=nc.gpsimd)
                aprod.pop(i)

            for i in range(1, nchunks):
                load_x(i)
            for i in range(nchunks - 2):
                phase_a(i)
                phase_b(i)
            # tail: A(n-2), A(n-1), B(n-2), B(n-1) so only one j-chain
            # trails the last matmul
            phase_a(nchunks - 2)
            phase_a(nchunks - 1)
            phase_b(nchunks - 2)
            phase_b(nchunks - 1)
            nc.sync.dma_start(clast_d, clast_sb[:])

    nc.compile()
    return nc


def _get_program():
    if "nc" not in _prog_cache:
        _prog_cache["nc"] = _build_program()
    return _prog_cache["nc"]


def _shard_inputs(x, W, b, c0):
    in_maps = []
    for core in range(NCORES):
        bi, hh = divmod(core, 2)
        xt = np.ascontiguousarray(x[bi].T)  # (D, S)
        rows = np.concatenate(
            [W[g * H + hh * HSH : g * H + (hh + 1) * HSH] for g in range(4)], axis=0
        )  # (4*HSH, D) ordered [ig; fg; og; cs]
        wt = np.ascontiguousarray(rows.T)  # (D, 4*HSH)
        bsh = np.concatenate(
            [b[g * H + hh * HSH : g * H + (hh + 1) * HSH] for g in range(4)]
        )  # (4*HSH,)
        bt = bsh.reshape(MT, 128).T  # [p, t]
        bias_arr = np.ascontiguousarray(
            np.concatenate([bt, -bt, bt + 0.5], axis=1), dtype=np.float32
        )  # (128, 3*MT)
        c0sh = c0[bi, 0, hh * HSH : (hh + 1) * HSH]
        c0_arr = np.ascontiguousarray(c0sh.reshape(JT, 128).T, dtype=np.float32)
        in_maps.append({"xt": xt, "wt": wt, "bias": bias_arr, "c0": c0_arr})
    return in_maps


def kernel(x, W, b, h0, c0, _trace=False):
    x = np.asarray(x, dtype=np.float32)
    W = np.asarray(W, dtype=np.float32)
    b = np.asarray(b, dtype=np.float32)
    c0 = np.asarray(c0, dtype=np.float32)

    nc = _get_program()
    in_maps = _shard_inputs(x, W, b, c0)
    try:
        res = run_bass_kernel_spmd(
            nc, in_maps, core_ids=list(range(NCORES)), trace=_trace
        )
    except (ModuleNotFoundError, ImportError):
        # NTFF profiling hooks unavailable in this environment
        res = run_bass_kernel_spmd(
            nc, in_maps, core_ids=list(range(NCORES)), trace=False
        )

    out = np.empty((B, S, H), dtype=np.float32)
    c_last = np.empty((B, 1, H), dtype=np.float32)
    for core in range(NCORES):
        bi, hh = divmod(core, 2)
        outT = res.results[core]["outT"]  # (HSH, S)
        out[bi, :, hh * HSH : (hh + 1) * HSH] = outT.T
        cl = res.results[core]["clast"]  # (128, JT): [p, j] -> h = j*128+p
        c_last[bi, 0, hh * HSH : (hh + 1) * HSH] = cl.T.reshape(-1)

    if _trace:
        kernel._last_results = res
    return out, out[:, -1:], c_last
